# revision 8
# baseline (speedup 1.0000x reference)
"""LeViT-style Attention2d block on 8 Trainium2 NeuronCores.

Data-parallel over batch B=256 (32 batches/core). Per core, per batch pair:
  - x^T staged as [DIM, tokens] so qkv matmuls contract DIM on partitions.
  - q/k computed "transposed" (channels on partitions, packed 4 heads x 32ch
    per 128-partition group) in fp32r at full PE rate (moving dim 392 >= 256).
  - v computed in natural [token, channel] layout (needed as AV lhsT), bf16.
  - Scores S^T[m, n] per head via K=32 row-tiled matmuls, bias added on DVE,
    exp on ACT -> bf16. PSUM discipline (device-fatal otherwise): a DVE read
    never spans two PSUM banks, and two matmuls share a bank only when they
    share a PE row group (which serializes them). Head h lives in q/k chunk
    h%2 at partition offset 32*(h//2), so a head pair (2i, 2i+1) shares row
    group i and one PSUM bank, while distinct pairs overlap 4-way.
  - Softmax denominators via an all-ones [m,128] lhsT matmul: one matmul
    yields the column sums replicated across all 128 partitions (no
    partition-broadcast op needed); 1/x via reciprocal_approx_fast.
  - AV in bf16 produces attnout^T [d, n]; normalize folded into the
    PSUM->SBUF copy (tensor_tensor mult), output declared fp32r.
  - Final projection contracts the 1024 channels in fp32r; bias (with the
    v-bias folded in, since softmax rows sum to 1) added on DVE; DMA out.

Numerics: q/k/v/P in bf16 for the attention core, fp32r for the two big
matmuls. k-bias is dropped (softmax-invariant row constant), v-bias is
folded into the projection bias.
"""

import numpy as np

import concourse.bacc as bacc
import concourse.mybir as mybir
import concourse.tile as tile
from concourse.bass_utils import run_bass_kernel_spmd

B, N_TOK, DIM = 256, 196, 384
H, KD, D, RES = 8, 32, 128, 14
NCORES = 8
BPC = B // NCORES          # 32 batches per core
NPAIRS = BPC // 2          # 16

F32 = mybir.dt.float32
F32R = mybir.dt.float32r
BF16 = mybir.dt.bfloat16
AF = mybir.ActivationFunctionType
ALU = mybir.AluOpType

_CACHE = {}


def _build_nc(loop_n=1):
    nc = bacc.Bacc("TRN2", target_bir_lowering=False, debug=False)
    xt_d = nc.dram_tensor("xt", [NPAIRS, 128, 3, 2, 196], F32R, kind="ExternalInput")
    wqk_d = nc.dram_tensor("wqk", [128, 3, 512], F32R, kind="ExternalInput")
    wv_d = nc.dram_tensor("wv", [128, 3, 1024], F32R, kind="ExternalInput")
    wpr_d = nc.dram_tensor("wpr", [128, 8, 384], F32R, kind="ExternalInput")
    bq_d = nc.dram_tensor("bq", [128, 2], F32, kind="ExternalInput")
    bprj_d = nc.dram_tensor("bprj", [128, 384], F32, kind="ExternalInput")
    biast_d = nc.dram_tensor("biast", [128, 2, 8, 196], F32, kind="ExternalInput")
    y_d = nc.dram_tensor("y", [BPC, 196, 384], F32, kind="ExternalOutput")

    with tile.TileContext(nc) as tc:
        with (
            tc.tile_pool(name="const", bufs=1) as cpool,
            tc.tile_pool(name="xin", bufs=3) as xpool,
            tc.tile_pool(name="qk", bufs=2) as qkpool,
            tc.tile_pool(name="vsb", bufs=4) as vpool,
            tc.tile_pool(name="stp", bufs=3) as stpool,
            tc.tile_pool(name="exp", bufs=4) as expool,
            tc.tile_pool(name="rsr", bufs=8) as rpool,
            tc.tile_pool(name="att", bufs=2) as apool,
            tc.tile_pool(name="yout", bufs=3) as ypool,
            tc.tile_pool(name="ps1", bufs=8, space="PSUM") as ps1,
        ):
            wqk_sb = cpool.tile([128, 3, 512], F32R)
            nc.sync.dma_start(wqk_sb, wqk_d[:])
            wv_sb = cpool.tile([128, 3, 1024], F32R)
            nc.sync.dma_start(wv_sb, wv_d[:])
            wpr_sb = cpool.tile([128, 8, 384], F32R)
            nc.sync.dma_start(wpr_sb, wpr_d[:])
            bq_sb = cpool.tile([128, 2], F32)
            nc.sync.dma_start(bq_sb, bq_d[:])
            bprj_sb = cpool.tile([128, 384], F32)
            nc.sync.dma_start(bprj_sb, bprj_d[:])
            biast_sb = cpool.tile([128, 2, 8, 196], F32)
            nc.sync.dma_start(biast_sb, biast_d[:])
            ones_sb = cpool.tile([128, 128], BF16)
            nc.vector.memset(ones_sb, 1.0)

            def do_pair(pair):
                xt_sb = xpool.tile([128, 3, 2, 196], F32R, tag="xt")
                nc.sync.dma_start(xt_sb, xt_d[pair])

                # ---- q/k (transposed, packed, bf16) ----
                qk_sb = qkpool.tile([128, 4, 2, 196], BF16, tag="qk")
                for g in range(4):
                    qs = ps1.tile([128, 2, 196], F32, tag="bank")
                    for kj in range(3):
                        nc.tensor.matmul(
                            qs[:],
                            wqk_sb[:, kj, 128 * g : 128 * (g + 1)],
                            xt_sb[:, kj],
                            start=(kj == 0),
                            stop=(kj == 2),
                        )
                    if g < 2:
                        nc.scalar.activation(
                            out=qk_sb[:, g], in_=qs[:], func=AF.Identity,
                            bias=bq_sb[:, g : g + 1],
                        )
                    else:
                        nc.scalar.activation(out=qk_sb[:, g], in_=qs[:], func=AF.Copy)

                # ---- v (natural layout, bf16) ----
                v_sbs = []
                for b2 in range(2):
                    v_sb = vpool.tile([128, 2, 1024], BF16, tag="v")
                    v_sbs.append(v_sb)
                    for mj in range(2):
                        rows = 128 if mj == 0 else 68
                        for nj in range(2):
                            vs = ps1.tile([128, 512], F32, tag="bank")
                            for kj in range(3):
                                nc.tensor.matmul(
                                    vs[:rows],
                                    xt_sb[:, kj, b2, 128 * mj : 128 * mj + rows],
                                    wv_sb[:, kj, 512 * nj : 512 * (nj + 1)],
                                    start=(kj == 0),
                                    stop=(kj == 2),
                                )
                            nc.any.tensor_copy(
                                out=v_sb[:rows, mj, 512 * nj : 512 * (nj + 1)],
                                in_=vs[:rows],
                            )

                for b2 in range(2):
                    b = 2 * pair + b2
                    # ---- scores + softmax numerator ----
                    expst = []
                    for j in range(2):
                        rows = 128 if j == 0 else 68
                        st_sb = stpool.tile([128, 8, 196], F32, tag="st")
                        for hp in range(4):
                            sp = ps1.tile([128, 2, 256], F32, tag="bank")
                            ro = 32 * hp
                            for hh in range(2):
                                nc.tensor.matmul(
                                    sp[:rows, hh, :196],
                                    qk_sb[ro : ro + 32, 2 + hh, b2,
                                          128 * j : 128 * j + rows],
                                    qk_sb[ro : ro + 32, hh, b2, :],
                                    start=True,
                                    stop=True,
                                    tile_position=(ro, 0),
                                )
                            nc.vector.tensor_tensor(
                                st_sb[:rows, 2 * hp : 2 * hp + 2, :],
                                sp[:rows, :, :196],
                                biast_sb[:rows, j, 2 * hp : 2 * hp + 2, :],
                                ALU.add,
                            )
                        ex = expool.tile([128, 8, 196], BF16, tag="expst")
                        nc.scalar.activation(out=ex[:rows], in_=st_sb[:rows], func=AF.Exp)
                        expst.append(ex)

                    # ---- softmax denominators, replicated across partitions ----
                    rsr = []
                    for hp in range(4):
                        cs = ps1.tile([128, 2, 196], F32, tag="bank")
                        for j in range(2):
                            rows = 128 if j == 0 else 68
                            nc.tensor.matmul(
                                cs[:],
                                ones_sb[:rows, :],
                                expst[j][:rows, 2 * hp : 2 * hp + 2, :],
                                start=(j == 0),
                                stop=(j == 1),
                            )
                        rr = rpool.tile([128, 2, 196], F32, tag="rsr")
                        nc.vector.reciprocal_approx_fast(out=rr[:], in_=cs[:])
                        rsr.append(rr)

                    # ---- AV (attnout^T) + fused normalize ----
                    attnT = apool.tile([128, 8, 196], F32R, tag="attnT")
                    for hp in range(4):
                        av = ps1.tile([128, 2, 196], F32, tag="bank")
                        for hh in range(2):
                            h = 2 * hp + hh
                            for j in range(2):
                                rows = 128 if j == 0 else 68
                                nc.tensor.matmul(
                                    av[:, hh, :],
                                    v_sbs[b2][:rows, j, 128 * h : 128 * (h + 1)],
                                    expst[j][:rows, h, :],
                                    start=(j == 0),
                                    stop=(j == 1),
                                )
                        nc.vector.tensor_tensor(
                            attnT[:, 2 * hp : 2 * hp + 2, :], av[:], rsr[hp][:], ALU.mult
                        )

                    # ---- projection ----
                    y_sb = ypool.tile([128, 2, 384], F32, tag="y")
                    for mj in range(2):
                        rows = 128 if mj == 0 else 68
                        pp = ps1.tile([128, 512], F32, tag="bank")
                        for h in range(8):
                            nc.tensor.matmul(
                                pp[:rows, :384],
                                attnT[:, h, 128 * mj : 128 * mj + rows],
                                wpr_sb[:, h, :],
                                start=(h == 0),
                                stop=(h == 7),
                            )
                        nc.vector.tensor_tensor(
                            y_sb[:rows, mj, :], pp[:rows, :384], bprj_sb[:rows, :], ALU.add
                        )
                    nc.sync.dma_start(y_d[b, 0:128, :], y_sb[:, 0, :])
                    nc.sync.dma_start(y_d[b, 128:196, :], y_sb[0:68, 1, :])

            if loop_n > 1:
                with tc.For_i(0, loop_n, 1):
                    for pair in range(NPAIRS):
                        do_pair(pair)
            else:
                for pair in range(NPAIRS):
                    do_pair(pair)
    nc.compile()
    return nc


def prep_inputs(x, Wqkv, bqkv, Wproj, bproj, ab_table, bias_idxs):
    """Host-side packing: weight layouts, bias folding, x transpose, sharding."""
    x = np.asarray(x, np.float32)
    Wqkv = np.asarray(Wqkv, np.float32)
    bqkv = np.asarray(bqkv, np.float32)
    Wproj = np.asarray(Wproj, np.float32)
    bproj = np.asarray(bproj, np.float32)
    ab_table = np.asarray(ab_table, np.float32)
    bias_idxs = np.asarray(bias_idxs)

    scale = KD ** -0.5
    rows_q = np.concatenate([np.arange(192 * h, 192 * h + KD) for h in range(H)])
    rows_k = rows_q + KD
    rows_v = np.concatenate([np.arange(192 * h + 2 * KD, 192 * h + 192) for h in range(H)])
    Wq = Wqkv[rows_q] * scale          # [256, 384], (head, ch)-major
    Wk = Wqkv[rows_k]
    Wv = Wqkv[rows_v]                  # [1024, 384]
    bq = bqkv[rows_q] * scale          # [256]
    bv = bqkv[rows_v]                  # [1024]

    # Head packing for the scores matmuls: head h lives in q/k chunk h%2 at
    # partition offset 32*(h//2), i.e. chunk 0 holds heads [0,2,4,6].
    perm = [0, 2, 4, 6, 1, 3, 5, 7]
    Wq = Wq.reshape(H, KD, DIM)[perm].reshape(H * KD, DIM)
    Wk = Wk.reshape(H, KD, DIM)[perm].reshape(H * KD, DIM)
    bq = bq.reshape(H, KD)[perm].reshape(H * KD)

    wqk_lhsT = np.concatenate([Wq[0:128], Wq[128:256], Wk[0:128], Wk[128:256]], axis=0).T
    wqk = np.ascontiguousarray(wqk_lhsT.reshape(3, 128, 512).transpose(1, 0, 2))
    wv_p = np.ascontiguousarray(Wv.T.reshape(3, 128, 1024).transpose(1, 0, 2))
    wpr = np.ascontiguousarray(Wproj.T.reshape(8, 128, 384).transpose(1, 0, 2))
    bq_pack = np.ascontiguousarray(bq.reshape(2, 128).T)           # [128, 2]
    bprj_eff = bproj + Wproj @ bv                                  # v-bias folded in
    bprj_rep = np.ascontiguousarray(np.broadcast_to(bprj_eff, (128, 384)))

    bias = ab_table[:, bias_idxs]                                  # [H, 196, 196]
    biast = np.zeros((128, 2, H, 196), np.float32)
    bias_mhn = bias.transpose(1, 0, 2)                             # [m, h, n]
    biast[:, 0] = bias_mhn[0:128]
    biast[0:68, 1] = bias_mhn[128:196]

    xt = x.transpose(0, 2, 1).reshape(B, 3, 128, 196)
    xt = xt.reshape(NCORES, NPAIRS, 2, 3, 128, 196).transpose(0, 1, 4, 3, 2, 5)
    xt = np.ascontiguousarray(xt, dtype=np.float32)                # [core, pair, p, j, b2, n]

    shared = {
        "wqk": wqk, "wv": wv_p, "wpr": wpr, "bq": bq_pack,
        "bprj": bprj_rep, "biast": biast,
    }
    in_maps = [dict(shared, xt=xt[c]) for c in range(NCORES)]
    return in_maps


def kernel(x, Wqkv, bqkv, Wproj, bproj, ab_table, bias_idxs):
    in_maps = prep_inputs(x, Wqkv, bqkv, Wproj, bproj, ab_table, bias_idxs)
    if "nc" not in _CACHE:
        _CACHE["nc"] = _build_nc()
    nc = _CACHE["nc"]
    res = run_bass_kernel_spmd(nc, in_maps, core_ids=list(range(NCORES)))
    out = np.concatenate([res.results[c]["y"] for c in range(NCORES)], axis=0)
    return out.astype(np.float32)
